# revision 1
# baseline (speedup 1.0000x reference)
"""Trainium2 Bass kernel for a dense transformer block (B=4,T=2048,C=1024,H=16).

Sharding: 8 cores, zero collectives. Core i handles batch i//2 and a
balanced half of the query tokens (i%2==0 -> chunks [0:512)+[1536:2048),
i%2==1 -> [512:1024)+[1024:1536)). All sharding is done on the host; the
device program is identical on every core (SPMD), only input data differs.

Per-core dataflow (tokens-on-free-axis for all matmul operands):
  LN1 (bn_stats, [tok,C] layout) -> h1 bf16 -> DRAM bounce -> DMA-transpose
  -> h1T [C, tok].  QKV in bf16: kT/qT stacked [H*64, tok] (2 heads per
  128-partition tile), V natural [tok, H*65] with a fused ones column so the
  PV matmul also produces the softmax denominator.  Scores are computed
  pre-transposed sT[tk, tq] = K Q^T so softmax needs no transpose of P and
  no max subtraction (scores are O(1)); exp runs on ACT straight from PSUM
  with the 1/sqrt(hd) scale fused.  Causality at 128-tile granularity with
  per-core host-built multiplicative masks (uniform program, per-core data).
  PV accumulates over tk tiles in PSUM; normalization = one reciprocal of
  the denominator row + gpsimd partition_broadcast + one multiply.
  proj and both FFN matmuls run in float32r (full-rate fp32).
"""

import sys
import numpy as np

for _p in ("/opt/trn_rl_repo", "/root/.axon_site/_ro/trn_rl_repo"):
    if _p not in sys.path:
        sys.path.append(_p)

import ml_dtypes  # noqa: E402
import concourse.bass as bass  # noqa: E402
import concourse.bacc as bacc  # noqa: E402
import concourse.tile as tile  # noqa: E402
from concourse import mybir  # noqa: E402
from concourse.bass_utils import run_bass_kernel_spmd  # noqa: E402
from concourse.masks import make_identity  # noqa: E402

B, T, C, H, HD = 4, 2048, 1024, 16, 64
NCORES = 8
EPS = 1e-5
F32 = mybir.dt.float32
F32R = mybir.dt.float32r
BF16 = mybir.dt.bfloat16
AF = mybir.ActivationFunctionType
ALU = mybir.AluOpType

_CACHE = {}

def _emit_body(nc, tc, io, ln1_triv, ln2_triv):
    # ---------------- long-lived pools ----------------
    def pool(name, bufs, space="SBUF"):
        cm = tc.tile_pool(name=name, bufs=bufs, space=space)
        p = cm.__enter__()
        return cm, p

    cm_singles, singles = pool("singles", 1)
    cm_ln, ln_pool = pool("ln", 3)
    cm_stat, stat_pool = pool("stat", 4)
    cm_small, small = pool("small", 2)
    cm_dram, dram = pool("dram", 1, "DRAM")

    eps_t = singles.tile([128, 1], F32, name="eps")
    nc.vector.memset(eps_t, EPS)
    ident = singles.tile([128, 128], F32, name="ident")
    make_identity(nc, ident)
    b1t_sb = singles.tile([128, 32], F32, name="b1t_sb")
    nc.sync.dma_start(out=b1t_sb, in_=io["b1t"])

    def bcast_ap(dram_ap):
        # [1024] dram vector -> [128,1024] partition-broadcast AP
        return bass.AP(
            tensor=dram_ap.tensor,
            offset=dram_ap.offset,
            ap=[[0, 128]] + list(dram_ap.ap),
        )

    bproj_sb = singles.tile([128, 1024], F32, name="bproj_sb")
    nc.gpsimd.dma_start(out=bproj_sb, in_=bcast_ap(io["b_proj"]))
    b2_sb = singles.tile([128, 1024], F32, name="b2_sb")
    nc.gpsimd.dma_start(out=b2_sb, in_=bcast_ap(io["b2"]))

    g1_sb = bb1_sb = g2_sb = bb2_sb = None
    if not ln1_triv:
        g1_sb = singles.tile([128, 1024], F32, name="g1_sb")
        nc.gpsimd.dma_start(out=g1_sb, in_=bcast_ap(io["ln1_g"]))
        bb1_sb = singles.tile([128, 1024], F32, name="bb1_sb")
        nc.gpsimd.dma_start(out=bb1_sb, in_=bcast_ap(io["ln1_b"]))
    if not ln2_triv:
        g2_sb = singles.tile([128, 1024], F32, name="g2_sb")
        nc.gpsimd.dma_start(out=g2_sb, in_=bcast_ap(io["ln2_g"]))
        bb2_sb = singles.tile([128, 1024], F32, name="bb2_sb")
        nc.gpsimd.dma_start(out=bb2_sb, in_=bcast_ap(io["ln2_b"]))

    # ---------------- LayerNorm helpers ----------------
    def ln_apply(xt, out_ap, trivial, g_sb, b_sb):
        st = stat_pool.tile([128, 2, 6], F32, tag="bnst", name="bnst")
        for sg in range(2):
            nc.vector.bn_stats(out=st[:, sg, :], in_=xt[:, sg * 512:(sg + 1) * 512])
        mv = stat_pool.tile([128, 2], F32, tag="bnmv", name="bnmv")
        nc.vector.bn_aggr(out=mv, in_=st)
        std = stat_pool.tile([128, 1], F32, tag="bnsd", name="bnsd")
        nc.scalar.activation(out=std, in_=mv[:, 1:2], func=AF.Sqrt, bias=eps_t,
                             scale=1.0)
        rstd = stat_pool.tile([128, 1], F32, tag="bnrs", name="bnrs")
        nc.vector.reciprocal(out=rstd, in_=std)
        if trivial:
            nc.vector.tensor_scalar(
                out=out_ap, in0=xt, scalar1=mv[:, 0:1], scalar2=rstd,
                op0=ALU.subtract, op1=ALU.mult)
        else:
            tmp = ln_pool.tile([128, 1024], F32, tag="lnx", name="lntmp")
            nc.vector.tensor_scalar(
                out=tmp, in0=xt, scalar1=mv[:, 0:1], scalar2=rstd,
                op0=ALU.subtract, op1=ALU.mult)
            nc.vector.tensor_mul(out=tmp, in0=tmp, in1=g_sb)
            nc.vector.tensor_add(out=out_ap, in0=tmp, in1=b_sb)

    def ln_tile(x_src_rows, out_ap, trivial, g_sb, b_sb):
        xt = ln_pool.tile([128, 1024], F32, tag="lnx", name="lnx")
        nc.sync.dma_start(out=xt, in_=x_src_rows)
        ln_apply(xt, out_ap, trivial, g_sb, b_sb)

    # long-lived result pools, opened bottom-of-stack (LIFO discipline)
    cm_kt, kt_pool = pool("kt", 8)
    cm_v, v_pool = pool("v", 16)
    cm_qt, qt_pool = pool("qt", 8)
    kT = [kt_pool.tile([128, 2048], BF16, tag="kt", name="kt") for _ in range(8)]
    Vt = [v_pool.tile([128, 16, 65], BF16, tag="vt", name="vt") for _ in range(16)]
    qT = [qt_pool.tile([128, 1024], BF16, tag="qt", name="qt") for _ in range(8)]
    x2d = dram.tile([1024, 1024], F32, name="x2d")

    # ---------------- Phase 1a: LN1 full batch -> h1 -> DRAM -> h1T ---------
    cm_h1tB, h1tB_pool = pool("h1tB", 8)
    cm_wk, wk_pool = pool("wk", 1)
    wkB = wk_pool.tile([128, 8, 1024], BF16, name="wkB")
    cm_wv, wv_pool = pool("wv", 1)
    wvB = wv_pool.tile([128, 8, 1024], BF16, name="wvB")
    cm_psq, ps_qkv = pool("ps_qkv", 4, "PSUM")
    cm_h1tA, h1tA_pool = pool("h1tA", 8)
    cm_h1, h1_pool = pool("h1", 3)

    h1d = dram.tile([2048, 1024], BF16, name="h1d")
    for t in range(16):
        ht = h1_pool.tile([128, 1024], BF16, tag="h1", name="h1")
        ln_tile(io["x_full"][t * 128:(t + 1) * 128, :], ht, ln1_triv, g1_sb, bb1_sb)
        nc.sync.dma_start(out=h1d[t * 128:(t + 1) * 128, :], in_=ht)
    h1TA = [h1tA_pool.tile([128, 1024], BF16, tag="h1tA", name="h1tA")
            for _ in range(8)]
    h1TB = [h1tB_pool.tile([128, 1024], BF16, tag="h1tB", name="h1tB")
            for _ in range(8)]
    for q in range(4):
        for c in range(8):
            dst = h1TA[c] if q < 2 else h1TB[c]
            nc.scalar.dma_start(
                out=dst[:, (q % 2) * 512:(q % 2 + 1) * 512],
                in_=h1d[q * 512:(q + 1) * 512, c * 128:(c + 1) * 128],
                transpose=True)
        if q == 0:
            for hh in range(2):
                nc.scalar.dma_start(
                    out=wkB[:, hh * 4:(hh + 1) * 4, :],
                    in_=io["wqk"][hh * 512:(hh + 1) * 512, 1024:2048]
                    .rearrange("(c p) n -> p c n", p=128))
        elif q == 1:
            for hh in range(2):
                nc.scalar.dma_start(
                    out=wvB[:, hh * 4:(hh + 1) * 4, :],
                    in_=io["wv"][hh * 512:(hh + 1) * 512, :]
                    .rearrange("(c p) n -> p c n", p=128))
    cm_h1.__exit__(None, None, None)

    def h1T(c, n):
        # transposed h1 slice [128, 512] for token chunk n (0..3)
        return (h1TA[c] if n < 2 else h1TB[c])[:, (n % 2) * 512:(n % 2 + 1) * 512]

    # ---------------- Phase 2: QKV ----------------

    def kt_unit(n, p, late=False):
        pp = ps_pv if late else ps_qkv
        ps = pp.tile([128, 512], F32, tag="pv" if late else "q", name="psk")
        for c in range(8):
            nc.tensor.matmul(
                out=ps, lhsT=wkB[:, c, p * 128:(p + 1) * 128],
                rhs=h1T(c, n), start=(c == 0), stop=(c == 7))
        nc.vector.tensor_copy(out=kT[p][:, n * 512:(n + 1) * 512], in_=ps)

    def v_unit(t, n, late=False):
        pp = ps_pv if late else ps_qkv
        ps = pp.tile([128, 512], F32, tag="pv" if late else "q", name="psv")
        for c in range(8):
            nc.tensor.matmul(
                out=ps, lhsT=(h1TA[c] if t < 8 else h1TB[c])
                [:, (t % 8) * 128:(t % 8 + 1) * 128],
                rhs=wvB[:, c, n * 512:(n + 1) * 512],
                start=(c == 0), stop=(c == 7))
        nc.vector.tensor_copy(
            out=Vt[t][:, n * 8:(n + 1) * 8, 0:64],
            in_=ps.rearrange("p (h d) -> p h d", d=64))
        if n == 1:
            nc.vector.memset(Vt[t][:, :, 64:65], 1.0)

    for q in range(2):          # consume transpose quarters greedily
        for p in range(8):
            kt_unit(q, p)
        for t in range(4 * q, 4 * q + 4):
            for n in range(2):
                v_unit(t, n)
    cm_h1tA.__exit__(None, None, None)

    # ---------------- Phase 1b + qT ----------------
    cm_h1th, h1th_pool = pool("h1th", 8)
    cm_h1b, h1b_pool = pool("h1b", 1)
    h1dh = dram.tile([1024, 1024], BF16, name="h1dh")
    for t in range(8):
        ht = h1b_pool.tile([128, 1024], BF16, tag="h1h", name="h1h")
        ln_tile(io["x_half"][t * 128:(t + 1) * 128, :], ht, ln1_triv, g1_sb, bb1_sb)
        nc.sync.dma_start(out=h1dh[t * 128:(t + 1) * 128, :], in_=ht)
    h1Th = [h1th_pool.tile([128, 1024], BF16, tag="h1th", name="h1th")
            for _ in range(8)]
    for q in range(2):
        for c in range(8):
            nc.scalar.dma_start(
                out=h1Th[c][:, q * 512:(q + 1) * 512],
                in_=h1dh[q * 512:(q + 1) * 512, c * 128:(c + 1) * 128],
                transpose=True)

    cm_wq, wq_pool = pool("wq", 1)
    wqB = wq_pool.tile([128, 8, 1024], BF16, name="wqB")
    nc.sync.dma_start(
        out=wqB, in_=io["wqk"][:, 0:1024].rearrange("(c p) n -> p c n", p=128))
    for n in range(2):
        for p in range(8):
            ps = ps_qkv.tile([128, 512], F32, tag="q", name="psq")
            for c in range(8):
                nc.tensor.matmul(
                    out=ps, lhsT=wqB[:, c, p * 128:(p + 1) * 128],
                    rhs=h1Th[c][:, n * 512:(n + 1) * 512],
                    start=(c == 0), stop=(c == 7))
            nc.vector.tensor_copy(out=qT[p][:, n * 512:(n + 1) * 512], in_=ps)
    cm_wq.__exit__(None, None, None)
    cm_h1b.__exit__(None, None, None)
    cm_h1th.__exit__(None, None, None)
    cm_psq.__exit__(None, None, None)

    # ---------------- Phase 3: attention (sw-pipelined, with fillers) -------
    cm_wp, wp_pool = pool("wproj", 1)
    wpB = wp_pool.tile([128, 8, 1024], BF16, name="wpB")
    nc.scalar.dma_start(
        out=wpB, in_=io["w_proj"].rearrange("(c p) n -> p c n", p=128))
    cm_att, att_pool = pool("attls", 3)

    cm_masks, masks_pool = pool("masks", 1)
    masks_sb = masks_pool.tile([128, 16, 512], BF16, name="masks_sb")
    nc.sync.dma_start(out=masks_sb, in_=io["masks"])

    cm_pt, pt_pool = pool("pt", 4)
    cm_ast, ast_pool = pool("attst", 4)
    cm_pssc, ps_sc = pool("ps_sc", 2, "PSUM")
    cm_pspv, ps_pv = pool("ps_pv", 4, "PSUM")

    attds = [dram.tile([1024, 512], BF16, name="attd")
             for _ in range(2)]  # [c=h*64+d, 512 tq] per slot
    SCALE = HD ** -0.5

    # filler work for slot0: remaining kT cols (n=2,3) and V tiles 8..15
    fillers = [("k", n, p) for n in (2, 3) for p in range(8)]
    fillers += [("v", t, n) for t in range(8, 16) for n in range(2)]
    fill_i = [0]

    def emit_filler():
        if fill_i[0] < len(fillers):
            kind, a, b = fillers[fill_i[0]]
            fill_i[0] += 1
            if kind == "k":
                kt_unit(a, b, late=True)
            else:
                v_unit(a, b, late=True)

    # proj units (t, n): t<4 available after slot0; t>=4 after slot1
    def proj_unit(t, n, acts, xh):
        ps = ps_pv.tile([128, 512], F32, tag="pv", name="psp")
        for c in range(8):
            nc.tensor.matmul(
                out=ps, lhsT=acts[:, c, :],
                rhs=wpB[:, c, n * 512:(n + 1) * 512],
                start=(c == 0), stop=(c == 7))
        sl = np.s_[:, n * 512:(n + 1) * 512]
        x2t = ln_pool.tile([128, 1024], F32, tag="lnx", name="x2t") \
            if n == 0 else proj_unit.x2t
        proj_unit.x2t = x2t
        nc.vector.tensor_add(out=x2t[sl], in0=ps, in1=xh[sl])
        nc.vector.tensor_add(out=x2t[sl], in0=x2t[sl], in1=bproj_sb[sl])
        if n == 1:
            nc.sync.dma_start(out=x2d[t * 128:(t + 1) * 128, :], in_=x2t)

    def load_proj_inputs(t):
        acts = att_pool.tile([128, 8, 128], BF16, tag="attls", name="attls")
        nc.scalar.dma_start(
            out=acts,
            in_=attds[t // 4][:, (t % 4) * 128:(t % 4 + 1) * 128]
            .rearrange("(c p) n -> p c n", p=128))
        xh = ln_pool.tile([128, 1024], F32, tag="lnx", name="xh2")
        nc.sync.dma_start(out=xh, in_=io["x_half"][t * 128:(t + 1) * 128, :])
        return acts, xh

    proj_jobs = []  # (t, acts, xh) pending per-slot1-hp emission

    def attn_slot(s, use_fillers, proj_ts):
        ntk = 8 if s == 0 else 16
        qc = s * 512
        for hp in range(8):
            pva = [ps_pv.tile([128, 512], F32, tag="pv", name="pv")
                   for _ in range(2)]
            pts = {}
            for tkt in range(ntk):
                ps = ps_sc.tile([128, 2, 512], F32, tag="sc", name="sc")
                for e in range(2):
                    nc.tensor.matmul(
                        out=ps[:, e, :],
                        lhsT=kT[hp][e * 64:(e + 1) * 64,
                                    tkt * 128:(tkt + 1) * 128],
                        rhs=qT[hp][e * 64:(e + 1) * 64, qc:qc + 512],
                        start=True, stop=True)
                pt = pt_pool.tile([128, 2, 512], BF16, tag="pt", name="pt")
                pts[tkt] = pt
                nc.scalar.activation(
                    out=pt.rearrange("p a b -> p (a b)"),
                    in_=ps.rearrange("p a b -> p (a b)"),
                    func=AF.Exp, scale=SCALE)
                if (s == 0) or (tkt >= 8):
                    for e in range(2):
                        nc.vector.tensor_mul(
                            out=pt[:, e, :], in0=pt[:, e, :],
                            in1=masks_sb[:, tkt, :])
                if tkt >= 1:
                    prev = pts.pop(tkt - 1)
                    for e in range(2):
                        nc.tensor.matmul(
                            out=pva[e][0:65, :],
                            lhsT=Vt[tkt - 1][:, 2 * hp + e, :],
                            rhs=prev[:, e, :],
                            start=(tkt - 1 == 0), stop=False)
                if use_fillers and tkt % 2 == 1:
                    emit_filler()
            last = pts.pop(ntk - 1)
            for e in range(2):
                nc.tensor.matmul(
                    out=pva[e][0:65, :],
                    lhsT=Vt[ntk - 1][:, 2 * hp + e, :],
                    rhs=last[:, e, :],
                    start=False, stop=True)
            for e in range(2):
                rec = small.tile([1, 512], F32, tag="rec", name="rec")
                nc.vector.reciprocal(out=rec, in_=pva[e][64:65, :])
                bc = small.tile([64, 512], F32, tag="bc", name="bc")
                nc.gpsimd.partition_broadcast(out_ap=bc, in_ap=rec)
                ast = ast_pool.tile([64, 512], BF16, tag="ast", name="ast")
                nc.vector.tensor_mul(out=ast, in0=pva[e][0:64, :], in1=bc)
                nc.sync.dma_start(
                    out=attds[s][hp * 128 + e * 64:hp * 128 + (e + 1) * 64, :],
                    in_=ast)
            if proj_ts and hp % 2 == 1:
                t = proj_ts[hp // 2]
                acts, xh = load_proj_inputs(t)
                for n in range(2):
                    proj_unit(t, n, acts, xh)

    attn_slot(0, True, None)
    while fill_i[0] < len(fillers):
        emit_filler()
    attn_slot(1, False, [0, 1, 2, 3])

    # proj t4..7
    for t in range(4, 8):
        acts, xh = load_proj_inputs(t)
        for n in range(2):
            proj_unit(t, n, acts, xh)

    cm_pspv.__exit__(None, None, None)
    cm_pssc.__exit__(None, None, None)
    cm_ast.__exit__(None, None, None)
    cm_pt.__exit__(None, None, None)
    cm_masks.__exit__(None, None, None)
    cm_att.__exit__(None, None, None)
    cm_wp.__exit__(None, None, None)
    cm_wv.__exit__(None, None, None)
    cm_wk.__exit__(None, None, None)
    cm_h1tB.__exit__(None, None, None)
    cm_qt.__exit__(None, None, None)
    cm_v.__exit__(None, None, None)
    cm_kt.__exit__(None, None, None)

    # ---------------- Phase 5: LN2 -> h2 -> h2T (PE transpose) --------------
    cm_psl, ps_late = pool("ps_late", 5, "PSUM")
    cm_h2t, h2t_pool = pool("h2t", 8)
    h2T = [h2t_pool.tile([128, 1024], F32R, tag="h2t", name="h2t")
           for _ in range(8)]
    for t in range(8):
        xt = ln_pool.tile([128, 1024], F32, tag="lnx", name="x2l")
        nc.sync.dma_start(out=xt, in_=x2d[t * 128:(t + 1) * 128, :])
        h2 = ln_pool.tile([128, 1024], F32, tag="lnx", name="h2")
        ln_apply(xt, h2, ln2_triv, g2_sb, bb2_sb)
        for c in range(8):
            pst = ps_late.tile([128, 128], F32, tag="l", name="pst")
            nc.tensor.transpose(out=pst, in_=h2[:, c * 128:(c + 1) * 128],
                                identity=ident)
            nc.scalar.copy(out=h2T[c][:, t * 128:(t + 1) * 128], in_=pst)

    # ---------------- Phase 6: FFN (2 passes x 4 j-blocks) ------------------
    cm_wb, wbig_pool = pool("wbig", 2)
    cm_rl, relu_pool = pool("relu", 1)
    cm_oa, oacc_pool = pool("oacc", 4)
    for pas in range(2):
        tok0 = pas * 512
        oacc = [oacc_pool.tile([128, 1024], F32, tag="oacc", name="oacc")
                for _ in range(4)]
        for jb in range(4):
            w1b = wbig_pool.tile([128, 8, 1024], F32R, tag="wb", name="w1b")
            for hh in range(4):
                nc.sync.dma_start(
                    out=w1b[:, hh * 2:(hh + 1) * 2, :],
                    in_=io["w1"][hh * 256:(hh + 1) * 256,
                                 jb * 1024:(jb + 1) * 1024]
                    .bitcast(F32R).rearrange("(c p) n -> p c n", p=128))
            relu_b = relu_pool.tile([128, 8, 512], F32R, tag="rl", name="rl")
            for j in range(8):
                ps = ps_late.tile([128, 512], F32, tag="l", name="psf1")
                for c in range(8):
                    nc.tensor.matmul(
                        out=ps,
                        lhsT=w1b[:, c, j * 128:(j + 1) * 128],
                        rhs=h2T[c][:, tok0:tok0 + 512],
                        start=(c == 0), stop=(c == 7))
                nc.scalar.activation(
                    out=relu_b[:, j, :], in_=ps, func=AF.Relu,
                    bias=b1t_sb[:, jb * 8 + j:jb * 8 + j + 1], scale=1.0)
            w2b = wbig_pool.tile([128, 8, 1024], F32R, tag="wb", name="w2b")
            for hh in range(4):
                nc.sync.dma_start(
                    out=w2b[:, hh * 2:(hh + 1) * 2, :],
                    in_=io["w2"][jb * 1024 + hh * 256:jb * 1024 + (hh + 1) * 256, :]
                    .bitcast(F32R).rearrange("(j p) n -> p j n", p=128))
            for tl in range(4):
                for n in range(2):
                    ps = ps_late.tile([128, 512], F32, tag="l", name="psf2")
                    for j in range(8):
                        nc.tensor.matmul(
                            out=ps,
                            lhsT=relu_b[:, j, tl * 128:(tl + 1) * 128],
                            rhs=w2b[:, j, n * 512:(n + 1) * 512],
                            start=(j == 0), stop=(j == 7))
                    sl = np.s_[:, n * 512:(n + 1) * 512]
                    if jb == 0:
                        nc.vector.tensor_copy(out=oacc[tl][sl], in_=ps)
                    else:
                        nc.vector.tensor_add(out=oacc[tl][sl], in0=oacc[tl][sl],
                                             in1=ps)
                if jb == 3:
                    tg = pas * 4 + tl
                    xld = ln_pool.tile([128, 1024], F32, tag="lnx", name="xld")
                    nc.sync.dma_start(out=xld,
                                      in_=x2d[tg * 128:(tg + 1) * 128, :])
                    nc.vector.tensor_add(out=oacc[tl], in0=oacc[tl], in1=xld)
                    nc.vector.tensor_add(out=oacc[tl], in0=oacc[tl], in1=b2_sb)
                    nc.sync.dma_start(out=io["out"][tg * 128:(tg + 1) * 128, :],
                                      in_=oacc[tl])

    cm_oa.__exit__(None, None, None)
    cm_rl.__exit__(None, None, None)
    cm_wb.__exit__(None, None, None)
    cm_h2t.__exit__(None, None, None)
    cm_psl.__exit__(None, None, None)
    cm_dram.__exit__(None, None, None)
    cm_small.__exit__(None, None, None)
    cm_stat.__exit__(None, None, None)
    cm_ln.__exit__(None, None, None)
    cm_singles.__exit__(None, None, None)


def build(ln1_triv=True, ln2_triv=True):
    key = (ln1_triv, ln2_triv)
    if key in _CACHE:
        return _CACHE[key]
    nc = bacc.Bacc("TRN2", target_bir_lowering=False, debug=False,
                   num_devices=NCORES)
    io = {}

    def din(name, shape, dt):
        io[name] = nc.dram_tensor(name, list(shape), dt, kind="ExternalInput").ap()

    din("x_full", (2048, 1024), F32)
    din("x_half", (1024, 1024), F32)
    din("wqk", (1024, 2048), BF16)
    din("wv", (1024, 1024), BF16)
    din("w_proj", (1024, 1024), BF16)
    din("b_proj", (1024,), F32)
    din("w1", (1024, 4096), F32)
    din("b1t", (128, 32), F32)
    din("w2", (4096, 1024), F32)
    din("b2", (1024,), F32)
    din("masks", (128, 16, 512), BF16)
    if not ln1_triv:
        din("ln1_g", (1024,), F32)
        din("ln1_b", (1024,), F32)
    if not ln2_triv:
        din("ln2_g", (1024,), F32)
        din("ln2_b", (1024,), F32)
    io["out"] = nc.dram_tensor("out", [1024, 1024], F32, kind="ExternalOutput").ap()

    with tile.TileContext(nc) as tc:
        _emit_body(nc, tc, io, ln1_triv, ln2_triv)
    nc.compile()
    _CACHE[key] = (nc, io)
    return nc, io


def _chunks(half):
    if half == 0:
        return (0, 1536)   # chunk A base, chunk B base
    return (512, 1024)


def _make_masks(half):
    """[128, 16, 512] bf16: m 0-7 = slot0 tiles (queries=chunkA),
    m 8-15 = slot1 tiles 8-15 (queries=chunkB)."""
    qa, qb = _chunks(half)
    out = np.zeros((128, 16, 512), np.float32)
    tk_l = np.arange(128)[:, None]
    tq_l = np.arange(512)[None, :]
    for m in range(8):
        out[:, m, :] = ((m * 128 + tk_l) <= (qa + tq_l))
    for m in range(8, 16):
        out[:, m, :] = ((m * 128 + tk_l) <= (qb + tq_l))
    return out.astype(ml_dtypes.bfloat16)


def _prep_common(inp, ln1_triv, ln2_triv):
    wq_f = np.ascontiguousarray(inp["wq"].transpose(1, 0, 2).reshape(C, C))
    wk_f = np.ascontiguousarray(inp["wk"].transpose(1, 0, 2).reshape(C, C))
    wv_f = np.ascontiguousarray(inp["wv"].transpose(1, 0, 2).reshape(C, C))
    wqk = np.concatenate([wq_f, wk_f], axis=1).astype(ml_dtypes.bfloat16)
    b1t = np.ascontiguousarray(inp["b1"].reshape(32, 128).T).astype(np.float32)
    common = {
        "wqk": wqk,
        "wv": wv_f.astype(ml_dtypes.bfloat16),
        "w_proj": inp["w_proj"].astype(ml_dtypes.bfloat16),
        "b_proj": inp["b_proj"].astype(np.float32),
        "w1": inp["w1"].astype(np.float32),
        "b1t": b1t,
        "w2": inp["w2"].astype(np.float32),
        "b2": inp["b2"].astype(np.float32),
    }
    if not ln1_triv:
        common["ln1_g"] = inp["ln1_g"].astype(np.float32)
        common["ln1_b"] = inp["ln1_b"].astype(np.float32)
    if not ln2_triv:
        common["ln2_g"] = inp["ln2_g"].astype(np.float32)
        common["ln2_b"] = inp["ln2_b"].astype(np.float32)
    return common


def make_in_maps(inputs):
    inp = {k: np.asarray(v) for k, v in inputs.items()}
    x = inp["x"].astype(np.float32)
    ln1_triv = bool(np.all(inp["ln1_g"] == 1.0) and np.all(inp["ln1_b"] == 0.0))
    ln2_triv = bool(np.all(inp["ln2_g"] == 1.0) and np.all(inp["ln2_b"] == 0.0))
    common = _prep_common(inp, ln1_triv, ln2_triv)
    in_maps = []
    for i in range(NCORES):
        b, half = i // 2, i % 2
        qa, qb = _chunks(half)
        xh = np.concatenate([x[b, qa:qa + 512], x[b, qb:qb + 512]], axis=0)
        m = dict(common)
        m["x_full"] = np.ascontiguousarray(x[b])
        m["x_half"] = np.ascontiguousarray(xh)
        m["masks"] = _make_masks(half)
        in_maps.append(m)
    return in_maps, ln1_triv, ln2_triv


def assemble(results):
    out = np.empty((B, T, C), np.float32)
    for i in range(NCORES):
        b, half = i // 2, i % 2
        qa, qb = _chunks(half)
        o = results[i]["out"]
        out[b, qa:qa + 512] = o[:512]
        out[b, qb:qb + 512] = o[512:]
    return out


def kernel(**inputs):
    in_maps, l1, l2 = make_in_maps(inputs)
    nc, io = build(l1, l2)
    res = run_bass_kernel_spmd(nc, in_maps, list(range(NCORES)))
    return assemble(res.results)


if __name__ == "__main__":
    build()
    print("build ok")



# revision 24
# speedup vs baseline: 42.4458x; 42.4458x over previous
"""Trainium2 Bass kernel for a dense transformer block (B=4,T=2048,C=1024,H=16).

Sharding: 8 cores, zero collectives. Core i handles batch i//2 and a
balanced half of the query tokens (i%2==0 -> chunks [0:512)+[1536:2048),
i%2==1 -> [512:1024)+[1024:1536)). All sharding is done on the host; the
device program is identical on every core (SPMD), only input data differs.

Per-core dataflow (tokens-on-free-axis for all matmul operands):
  LN1 (bn_stats, [tok,C] layout) -> h1 bf16 -> DRAM bounce -> DMA-transpose
  -> h1T [C, tok].  QKV in bf16: kT/qT stacked [H*64, tok] (2 heads per
  128-partition tile), V natural [tok, H*65] with a fused ones column so the
  PV matmul also produces the softmax denominator.  Scores are computed
  pre-transposed sT[tk, tq] = K Q^T so softmax needs no transpose of P and
  no max subtraction (scores are O(1)); exp runs on ACT straight from PSUM
  with the 1/sqrt(hd) scale fused.  Causality at 128-tile granularity with
  per-core host-built multiplicative masks (uniform program, per-core data).
  PV accumulates over tk tiles in PSUM; normalization = one reciprocal of
  the denominator row + gpsimd partition_broadcast + one multiply.
  proj and both FFN matmuls run in float32r (full-rate fp32).
"""

import sys
import numpy as np

for _p in ("/opt/trn_rl_repo", "/root/.axon_site/_ro/trn_rl_repo"):
    if _p not in sys.path:
        sys.path.append(_p)

import ml_dtypes  # noqa: E402
import concourse.bass as bass  # noqa: E402
import concourse.bacc as bacc  # noqa: E402
import concourse.tile as tile  # noqa: E402
from concourse import mybir  # noqa: E402
from concourse.bass_utils import run_bass_kernel_spmd  # noqa: E402
from concourse.masks import make_identity  # noqa: E402

B, T, C, H, HD = 4, 2048, 1024, 16, 64
NCORES = 8
EPS = 1e-5
F32 = mybir.dt.float32
F32R = mybir.dt.float32r
BF16 = mybir.dt.bfloat16
AF = mybir.ActivationFunctionType
ALU = mybir.AluOpType

_CACHE = {}

def _emit_body(nc, tc, io, ln1_triv, ln2_triv):
    # ---------------- long-lived pools ----------------
    def pool(name, bufs, space="SBUF"):
        cm = tc.tile_pool(name=name, bufs=bufs, space=space)
        p = cm.__enter__()
        return cm, p

    cm_singles, singles = pool("singles", 1)
    cm_ln, ln_pool = pool("ln", 3)
    cm_stat, stat_pool = pool("stat", 4)
    cm_small, small = pool("small", 2)
    cm_dram, dram = pool("dram", 1, "DRAM")

    eps_t = singles.tile([128, 1], F32, name="eps")
    nc.vector.memset(eps_t, EPS)
    ident_bf = singles.tile([128, 128], BF16, name="ident_bf")
    make_identity(nc, ident_bf)
    b1t_sb = singles.tile([128, 32], F32, name="b1t_sb")
    nc.sync.dma_start(out=b1t_sb, in_=io["b1t"])

    def bcast_ap(dram_ap):
        # [1024] dram vector -> [128,1024] partition-broadcast AP
        return bass.AP(
            tensor=dram_ap.tensor,
            offset=dram_ap.offset,
            ap=[[0, 128]] + list(dram_ap.ap),
        )

    bproj_sb = singles.tile([128, 1024], F32, name="bproj_sb")
    nc.gpsimd.dma_start(out=bproj_sb, in_=bcast_ap(io["b_proj"]))

    g1_sb = bb1_sb = g2_sb = bb2_sb = None
    if not ln1_triv:
        g1_sb = singles.tile([128, 1024], F32, name="g1_sb")
        nc.gpsimd.dma_start(out=g1_sb, in_=bcast_ap(io["ln1_g"]))
        bb1_sb = singles.tile([128, 1024], F32, name="bb1_sb")
        nc.gpsimd.dma_start(out=bb1_sb, in_=bcast_ap(io["ln1_b"]))
    if not ln2_triv:
        g2_sb = singles.tile([128, 1024], F32, name="g2_sb")
        nc.gpsimd.dma_start(out=g2_sb, in_=bcast_ap(io["ln2_g"]))
        bb2_sb = singles.tile([128, 1024], F32, name="bb2_sb")
        nc.gpsimd.dma_start(out=bb2_sb, in_=bcast_ap(io["ln2_b"]))

    # ---------------- LayerNorm helpers ----------------
    def ln_apply(xt, out_ap, trivial, g_sb, b_sb):
        st = stat_pool.tile([128, 2, 6], F32, tag="bnst", name="bnst")
        for sg in range(2):
            nc.vector.bn_stats(out=st[:, sg, :], in_=xt[:, sg * 512:(sg + 1) * 512])
        mv = stat_pool.tile([128, 2], F32, tag="bnmv", name="bnmv")
        nc.vector.bn_aggr(out=mv, in_=st)
        std = stat_pool.tile([128, 1], F32, tag="bnsd", name="bnsd")
        nc.scalar.activation(out=std, in_=mv[:, 1:2], func=AF.Sqrt, bias=eps_t,
                             scale=1.0)
        rstd = stat_pool.tile([128, 1], F32, tag="bnrs", name="bnrs")
        nc.vector.reciprocal(out=rstd, in_=std)
        if trivial:
            nc.vector.tensor_scalar(
                out=out_ap, in0=xt, scalar1=mv[:, 0:1], scalar2=rstd,
                op0=ALU.subtract, op1=ALU.mult)
        else:
            tmp = ln_pool.tile([128, 1024], F32, tag="lnx", name="lntmp")
            nc.vector.tensor_scalar(
                out=tmp, in0=xt, scalar1=mv[:, 0:1], scalar2=rstd,
                op0=ALU.subtract, op1=ALU.mult)
            nc.vector.tensor_mul(out=tmp, in0=tmp, in1=g_sb)
            nc.vector.tensor_add(out=out_ap, in0=tmp, in1=b_sb)

    def ln_tile(x_src_rows, out_ap, trivial, g_sb, b_sb):
        xt = ln_pool.tile([128, 1024], F32, tag="lnx", name="lnx")
        nc.sync.dma_start(out=xt, in_=x_src_rows)
        ln_apply(xt, out_ap, trivial, g_sb, b_sb)

    # long-lived result pools, opened bottom-of-stack (LIFO discipline)
    cm_kt, kt_pool = pool("kt", 8)
    cm_v, v_pool = pool("v", 16)
    cm_qt, qt_pool = pool("qt", 8)
    kT = [kt_pool.tile([128, 2048], BF16, tag="kt", name="kt") for _ in range(8)]
    Vt = [v_pool.tile([128, 16, 65], BF16, tag="vt", name="vt") for _ in range(16)]
    qT = [qt_pool.tile([128, 1024], BF16, tag="qt", name="qt") for _ in range(8)]
    x2d = dram.tile([1024, 1024], F32, name="x2d")

    # ---------------- Phase 1+2: LN1 -> PE-transpose -> QKV -----------------
    # weights stream on the scalar queue while x tiles stream on sync
    cm_wqkv, wqkv_pool = pool("wqkv", 2)
    wkB = wqkv_pool.tile([128, 8, 1024], BF16, tag="w", name="wkB")
    nc.scalar.dma_start(
        out=wkB, in_=io["wqk"][:, 1024:2048].rearrange("(c p) n -> p c n", p=128))
    wvB = wqkv_pool.tile([128, 8, 1024], BF16, tag="w", name="wvB")
    nc.scalar.dma_start(
        out=wvB, in_=io["wv"].rearrange("(c p) n -> p c n", p=128))

    cm_h1t, h1t_pool = pool("h1t", 2)
    cm_pst, ps_tr = pool("ps_tr", 4, "PSUM")
    cm_psq, ps_qkv = pool("ps_qkv", 4, "PSUM")
    cm_h1, h1_pool = pool("h1", 3)

    def ln_transpose(src_rows, dstT, col):
        ht = h1_pool.tile([128, 1024], BF16, tag="h1", name="h1")
        ln_tile(src_rows, ht, ln1_triv, g1_sb, bb1_sb)
        for g in range(2):
            pst = ps_tr.tile([128, 4, 128], BF16, tag="tr", name="pst")
            for c4 in range(4):
                nc.tensor.transpose(
                    out=pst[:, c4, :],
                    in_=ht[:, (g * 4 + c4) * 128:(g * 4 + c4 + 1) * 128],
                    identity=ident_bf)
            nc.scalar.copy(
                out=dstT[:, g * 4:(g + 1) * 4, col * 128:(col + 1) * 128],
                in_=pst)

    h1TA = h1t_pool.tile([128, 8, 1024], BF16, tag="h1t", name="h1TA")
    h1TB = h1t_pool.tile([128, 8, 1024], BF16, tag="h1t", name="h1TB")

    def h1T(c, n):
        # transposed h1 slice [128, 512] for token chunk n (0..3)
        src = h1TA if n < 2 else h1TB
        return src[:, c, (n % 2) * 512:(n % 2 + 1) * 512]

    def h1Tt(c, t):
        # transposed h1 slice [128, 128] for token tile t (0..15)
        src = h1TA if t < 8 else h1TB
        return src[:, c, (t % 8) * 128:(t % 8 + 1) * 128]

    def kt_unit(n, p, late=False):
        pp = ps_pv if late else ps_qkv
        ps = pp.tile([128, 512], F32, tag="pv" if late else "q", name="psk")
        for c in range(8):
            nc.tensor.matmul(
                out=ps, lhsT=wkB[:, c, p * 128:(p + 1) * 128],
                rhs=h1T(c, n), start=(c == 0), stop=(c == 7))
        nc.vector.tensor_copy(out=kT[p][:, n * 512:(n + 1) * 512], in_=ps)

    def v_unit(t, n, late=False):
        pp = ps_pv if late else ps_qkv
        ps = pp.tile([128, 512], F32, tag="pv" if late else "q", name="psv")
        for c in range(8):
            nc.tensor.matmul(
                out=ps, lhsT=h1Tt(c, t),
                rhs=wvB[:, c, n * 512:(n + 1) * 512],
                start=(c == 0), stop=(c == 7))
        nc.vector.tensor_copy(
            out=Vt[t][:, n * 8:(n + 1) * 8, 0:64],
            in_=ps.rearrange("p (h d) -> p h d", d=64))
        if n == 1:
            nc.vector.memset(Vt[t][:, :, 64:65], 1.0)

    def q_unit(n, p, late=False):
        pp = ps_pv if late else ps_qkv
        ps = pp.tile([128, 512], F32, tag="pv" if late else "q", name="psq")
        for c in range(8):
            nc.tensor.matmul(
                out=ps, lhsT=wqB[:, c, p * 128:(p + 1) * 128],
                rhs=h1Th[:, c, n * 512:(n + 1) * 512],
                start=(c == 0), stop=(c == 7))
        nc.vector.tensor_copy(out=qT[p][:, n * 512:(n + 1) * 512], in_=ps)

    # A half (tokens 0-1023): LN+transpose, then K(n=0,1) and V tiles 0-7
    for t in range(8):
        ln_transpose(io["x_full"][t * 128:(t + 1) * 128, :], h1TA, t)
        if 4 <= t:
            v_unit(t - 4, 0)
            v_unit(t - 4, 1)
        if t == 7:
            for p in range(8):
                kt_unit(0, p)
    for p in range(8):
        kt_unit(1, p)
    for t in range(4, 8):
        v_unit(t, 0)
        v_unit(t, 1)
    # B half (tokens 1024-2047): transposes, then K(n=2,3)
    for t in range(8, 16):
        ln_transpose(io["x_full"][t * 128:(t + 1) * 128, :], h1TB, t - 8)
        if t == 15:
            for p in range(8):
                kt_unit(2, p)
    for p in range(8):
        kt_unit(3, p)
    # queries: LN+transpose of x_half, wq reuses wk's pool slot
    h1Th = h1t_pool.tile([128, 8, 1024], BF16, tag="h1t", name="h1Th")
    wqB = wqkv_pool.tile([128, 8, 1024], BF16, tag="w", name="wqB")
    nc.scalar.dma_start(
        out=wqB, in_=io["wqk"][:, 0:1024].rearrange("(c p) n -> p c n", p=128))
    for th in range(8):
        ln_transpose(io["x_half"][th * 128:(th + 1) * 128, :], h1Th, th)
    for p in range(8):
        q_unit(0, p)
    cm_h1.__exit__(None, None, None)
    cm_psq.__exit__(None, None, None)
    cm_pst.__exit__(None, None, None)

    # ---------------- Phase 3: attention (sw-pipelined, with fillers) -------
    cm_wp, wp_pool = pool("wproj", 1)
    wpB = wp_pool.tile([128, 8, 1024], BF16, name="wpB")
    nc.scalar.dma_start(
        out=wpB, in_=io["w_proj"].rearrange("(c p) n -> p c n", p=128))
    cm_att, att_pool = pool("attls", 2)

    cm_masks, masks_pool = pool("masks", 1)

    def load_masks(s):
        m = masks_pool.tile([128, 8, 512], BF16, tag="m", name="masks_sb")
        nc.sync.dma_start(out=m, in_=io["masks"][:, s * 8:(s + 1) * 8, :])
        return m

    cm_pt, pt_pool = pool("pt", 3)
    cm_ast, ast_pool = pool("attst", 2)
    cm_pssc, ps_sc = pool("ps_sc", 2, "PSUM")
    cm_pspv, ps_pv = pool("ps_pv", 4, "PSUM")

    attds = [dram.tile([1024, 512], BF16, name="attd")
             for _ in range(2)]  # [c=h*64+d, 512 tq] per slot
    SCALE = HD ** -0.5

    # filler work for slot0: V tiles 8..15 and the slot1 queries (qT n=1)
    fillers = [("v", t, n) for t in range(8, 16) for n in range(2)]
    fillers += [("q", 1, p) for p in range(8)]
    fill_i = [0]

    def emit_filler():
        if fill_i[0] < len(fillers):
            kind, a, b = fillers[fill_i[0]]
            fill_i[0] += 1
            if kind == "v":
                v_unit(a, b, late=True)
            else:
                q_unit(a, b, late=True)

    # proj units (t, n): t<4 available after slot0; t>=4 after slot1
    def proj_unit(t, n, acts, xh):
        ps = ps_pv.tile([128, 512], F32, tag="pv", name="psp")
        for c in range(8):
            nc.tensor.matmul(
                out=ps, lhsT=acts[:, c, :],
                rhs=wpB[:, c, n * 512:(n + 1) * 512],
                start=(c == 0), stop=(c == 7))
        sl = np.s_[:, n * 512:(n + 1) * 512]
        x2t = ln_pool.tile([128, 1024], F32, tag="lnx", name="x2t") \
            if n == 0 else proj_unit.x2t
        proj_unit.x2t = x2t
        nc.vector.tensor_add(out=x2t[sl], in0=ps, in1=xh[sl])
        nc.vector.tensor_add(out=x2t[sl], in0=x2t[sl], in1=bproj_sb[sl])
        if n == 1:
            nc.sync.dma_start(out=x2d[t * 128:(t + 1) * 128, :], in_=x2t)

    def load_proj_inputs(t):
        acts = att_pool.tile([128, 8, 128], BF16, tag="attls", name="attls")
        nc.scalar.dma_start(
            out=acts,
            in_=attds[t // 4][:, (t % 4) * 128:(t % 4 + 1) * 128]
            .rearrange("(c p) n -> p c n", p=128))
        xh = ln_pool.tile([128, 1024], F32, tag="lnx", name="xh2")
        nc.sync.dma_start(out=xh, in_=io["x_half"][t * 128:(t + 1) * 128, :])
        return acts, xh

    proj_jobs = []  # (t, acts, xh) pending per-slot1-hp emission

    def attn_slot(s, msb, use_fillers, proj_ts):
        ntk = 8 if s == 0 else 16
        qc = s * 512
        for hp in range(8):
            pva = [ps_pv.tile([128, 512], F32, tag="pv", name="pv")
                   for _ in range(2)]
            pts = {}
            for tkt in range(ntk):
                ps = ps_sc.tile([128, 2, 512], F32, tag="sc", name="sc")
                for e in range(2):
                    nc.tensor.matmul(
                        out=ps[:, e, :],
                        lhsT=kT[hp][e * 64:(e + 1) * 64,
                                    tkt * 128:(tkt + 1) * 128],
                        rhs=qT[hp][e * 64:(e + 1) * 64, qc:qc + 512],
                        start=True, stop=True)
                pt = pt_pool.tile([128, 2, 512], BF16, tag="pt", name="pt")
                pts[tkt] = pt
                nc.scalar.activation(
                    out=pt.rearrange("p a b -> p (a b)"),
                    in_=ps.rearrange("p a b -> p (a b)"),
                    func=AF.Exp, scale=SCALE)
                if (s == 0) or (tkt >= 8):
                    for e in range(2):
                        nc.vector.tensor_mul(
                            out=pt[:, e, :], in0=pt[:, e, :],
                            in1=msb[:, tkt - 8 * s, :])
                if tkt >= 1:
                    prev = pts.pop(tkt - 1)
                    for e in range(2):
                        nc.tensor.matmul(
                            out=pva[e][0:65, :],
                            lhsT=Vt[tkt - 1][:, 2 * hp + e, :],
                            rhs=prev[:, e, :],
                            start=(tkt - 1 == 0), stop=False)
                if use_fillers and tkt % 2 == 1:
                    emit_filler()
            last = pts.pop(ntk - 1)
            for e in range(2):
                nc.tensor.matmul(
                    out=pva[e][0:65, :],
                    lhsT=Vt[ntk - 1][:, 2 * hp + e, :],
                    rhs=last[:, e, :],
                    start=False, stop=True)
            for e in range(2):
                rec = small.tile([1, 512], BF16, tag="rec", name="rec")
                with nc.allow_low_precision(reason="softmax denom recip bf16"):
                    nc.vector.reciprocal(out=rec, in_=pva[e][64:65, :])
                bc = small.tile([64, 512], BF16, tag="bc", name="bc")
                nc.gpsimd.partition_broadcast(out_ap=bc, in_ap=rec)
                ast = ast_pool.tile([64, 512], BF16, tag="ast", name="ast")
                nc.vector.tensor_mul(out=ast, in0=pva[e][0:64, :], in1=bc)
                nc.sync.dma_start(
                    out=attds[s][hp * 128 + e * 64:hp * 128 + (e + 1) * 64, :],
                    in_=ast)
            if proj_ts and hp % 2 == 1:
                t = proj_ts[hp // 2]
                acts, xh = load_proj_inputs(t)
                for n in range(2):
                    proj_unit(t, n, acts, xh)

    attn_slot(0, load_masks(0), True, None)
    while fill_i[0] < len(fillers):
        emit_filler()
    attn_slot(1, load_masks(1), False, [0, 1, 2, 3])

    # proj t4..7
    for t in range(4, 8):
        acts, xh = load_proj_inputs(t)
        for n in range(2):
            proj_unit(t, n, acts, xh)

    cm_pspv.__exit__(None, None, None)
    cm_pssc.__exit__(None, None, None)
    cm_ast.__exit__(None, None, None)
    cm_pt.__exit__(None, None, None)
    cm_masks.__exit__(None, None, None)
    cm_att.__exit__(None, None, None)
    cm_wp.__exit__(None, None, None)
    cm_h1t.__exit__(None, None, None)
    cm_wqkv.__exit__(None, None, None)
    cm_qt.__exit__(None, None, None)
    cm_v.__exit__(None, None, None)
    cm_kt.__exit__(None, None, None)

    # ---------------- Phase 5: LN2 -> h2 -> h2T (PE transpose) --------------
    # open the FFN weight pool first so the first-jb weight loads can be
    # issued before LN2 and overlap it
    cm_ls, late_singles = pool("lsing", 1)
    b2_sb = late_singles.tile([128, 1024], F32, name="b2_sb")
    nc.gpsimd.dma_start(out=b2_sb, in_=bcast_ap(io["b2"]))
    cm_wb, wbig_pool = pool("wbig", 3)

    def load_w1b(jb):
        w1b = wbig_pool.tile([128, 8, 1024], BF16, tag="wb", name="w1b")
        for hh in range(4):
            nc.sync.dma_start(
                out=w1b[:, hh * 2:(hh + 1) * 2, :],
                in_=io["w1"][hh * 256:(hh + 1) * 256,
                             jb * 1024:(jb + 1) * 1024]
                .rearrange("(c p) n -> p c n", p=128))
        return w1b

    def load_w2b(jb):
        w2b = wbig_pool.tile([128, 8, 1024], BF16, tag="wb", name="w2b")
        for hh in range(4):
            nc.sync.dma_start(
                out=w2b[:, hh * 2:(hh + 1) * 2, :],
                in_=io["w2"][jb * 1024 + hh * 256:jb * 1024 + (hh + 1) * 256, :]
                .rearrange("(j p) n -> p j n", p=128))
        return w2b

    w1b_next = load_w1b(0)
    w2b_next = load_w2b(0)

    cm_psl, ps_late = pool("ps_late", 5, "PSUM")
    cm_h2t, h2t_pool = pool("h2t", 8)
    h2T = [h2t_pool.tile([128, 1024], BF16, tag="h2t", name="h2t")
           for _ in range(8)]
    for t in range(8):
        xt = ln_pool.tile([128, 1024], F32, tag="lnx", name="x2l")
        nc.sync.dma_start(out=xt, in_=x2d[t * 128:(t + 1) * 128, :])
        h2 = ln_pool.tile([128, 1024], BF16, tag="lnx", name="h2")
        ln_apply(xt, h2, ln2_triv, g2_sb, bb2_sb)
        for c in range(8):
            pst = ps_late.tile([128, 128], BF16, tag="l", name="pst")
            nc.tensor.transpose(out=pst, in_=h2[:, c * 128:(c + 1) * 128],
                                identity=ident_bf)
            nc.scalar.copy(out=h2T[c][:, t * 128:(t + 1) * 128], in_=pst)

    # ---------------- Phase 6: FFN (4 j-blocks, weights loaded once) --------
    cm_rl, relu_pool = pool("relu", 2)
    cm_oa, oacc_pool = pool("oacc", 8)
    oacc = [oacc_pool.tile([128, 1024], F32, tag="oacc", name="oacc")
            for _ in range(8)]
    for jb in range(4):
        w1b = w1b_next
        relu_b = relu_pool.tile([128, 8, 2, 512], BF16, tag="rl", name="rl")
        for pas in range(2):
            tok0 = pas * 512
            for j in range(8):
                ps = ps_late.tile([128, 512], F32, tag="l", name="psf1")
                for c in range(8):
                    nc.tensor.matmul(
                        out=ps,
                        lhsT=w1b[:, c, j * 128:(j + 1) * 128],
                        rhs=h2T[c][:, tok0:tok0 + 512],
                        start=(c == 0), stop=(c == 7))
                nc.scalar.activation(
                    out=relu_b[:, j, pas, :], in_=ps, func=AF.Relu,
                    bias=b1t_sb[:, jb * 8 + j:jb * 8 + j + 1], scale=1.0)
        w2b = w2b_next
        if jb < 3:
            w1b_next = load_w1b(jb + 1)
        for pas in range(2):
            for tl in range(4):
                tg = pas * 4 + tl
                for n in range(2):
                    ps = ps_late.tile([128, 512], F32, tag="l", name="psf2")
                    for j in range(8):
                        nc.tensor.matmul(
                            out=ps,
                            lhsT=relu_b[:, j, pas, tl * 128:(tl + 1) * 128],
                            rhs=w2b[:, j, n * 512:(n + 1) * 512],
                            start=(j == 0), stop=(j == 7))
                    sl = np.s_[:, n * 512:(n + 1) * 512]
                    if jb == 0:
                        nc.vector.tensor_copy(out=oacc[tg][sl], in_=ps)
                    else:
                        nc.vector.tensor_add(out=oacc[tg][sl], in0=oacc[tg][sl],
                                             in1=ps)
                if jb == 3:
                    xld = ln_pool.tile([128, 1024], F32, tag="lnx", name="xld")
                    nc.sync.dma_start(out=xld,
                                      in_=x2d[tg * 128:(tg + 1) * 128, :])
                    nc.vector.tensor_add(out=oacc[tg], in0=oacc[tg], in1=xld)
                    nc.vector.tensor_add(out=oacc[tg], in0=oacc[tg], in1=b2_sb)
                    nc.sync.dma_start(out=io["out"][tg * 128:(tg + 1) * 128, :],
                                      in_=oacc[tg])
            if jb < 3 and pas == 0:
                w2b_next = load_w2b(jb + 1)

    cm_oa.__exit__(None, None, None)
    cm_rl.__exit__(None, None, None)
    cm_h2t.__exit__(None, None, None)
    cm_psl.__exit__(None, None, None)
    cm_wb.__exit__(None, None, None)
    cm_ls.__exit__(None, None, None)
    cm_dram.__exit__(None, None, None)
    cm_small.__exit__(None, None, None)
    cm_stat.__exit__(None, None, None)
    cm_ln.__exit__(None, None, None)
    cm_singles.__exit__(None, None, None)


def build(ln1_triv=True, ln2_triv=True):
    key = (ln1_triv, ln2_triv)
    if key in _CACHE:
        return _CACHE[key]
    nc = bacc.Bacc("TRN2", target_bir_lowering=False, debug=False,
                   num_devices=NCORES)
    io = {}

    def din(name, shape, dt):
        io[name] = nc.dram_tensor(name, list(shape), dt, kind="ExternalInput").ap()

    din("x_full", (2048, 1024), F32)
    din("x_half", (1024, 1024), F32)
    din("wqk", (1024, 2048), BF16)
    din("wv", (1024, 1024), BF16)
    din("w_proj", (1024, 1024), BF16)
    din("b_proj", (1024,), F32)
    din("w1", (1024, 4096), BF16)
    din("b1t", (128, 32), F32)
    din("w2", (4096, 1024), BF16)
    din("b2", (1024,), F32)
    din("masks", (128, 16, 512), BF16)
    if not ln1_triv:
        din("ln1_g", (1024,), F32)
        din("ln1_b", (1024,), F32)
    if not ln2_triv:
        din("ln2_g", (1024,), F32)
        din("ln2_b", (1024,), F32)
    io["out"] = nc.dram_tensor("out", [1024, 1024], F32, kind="ExternalOutput").ap()

    with tile.TileContext(nc) as tc:
        _emit_body(nc, tc, io, ln1_triv, ln2_triv)
    nc.compile()
    _CACHE[key] = (nc, io)
    return nc, io


def _chunks(half):
    if half == 0:
        return (0, 1536)   # chunk A base, chunk B base
    return (512, 1024)


def _make_masks(half):
    """[128, 16, 512] bf16: m 0-7 = slot0 tiles (queries=chunkA),
    m 8-15 = slot1 tiles 8-15 (queries=chunkB)."""
    qa, qb = _chunks(half)
    out = np.zeros((128, 16, 512), np.float32)
    tk_l = np.arange(128)[:, None]
    tq_l = np.arange(512)[None, :]
    for m in range(8):
        out[:, m, :] = ((m * 128 + tk_l) <= (qa + tq_l))
    for m in range(8, 16):
        out[:, m, :] = ((m * 128 + tk_l) <= (qb + tq_l))
    return out.astype(ml_dtypes.bfloat16)


def _prep_common(inp, ln1_triv, ln2_triv):
    wq_f = np.ascontiguousarray(inp["wq"].transpose(1, 0, 2).reshape(C, C))
    wk_f = np.ascontiguousarray(inp["wk"].transpose(1, 0, 2).reshape(C, C))
    wv_f = np.ascontiguousarray(inp["wv"].transpose(1, 0, 2).reshape(C, C))
    wqk = np.concatenate([wq_f, wk_f], axis=1).astype(ml_dtypes.bfloat16)
    b1t = np.ascontiguousarray(inp["b1"].reshape(32, 128).T).astype(np.float32)
    common = {
        "wqk": wqk,
        "wv": wv_f.astype(ml_dtypes.bfloat16),
        "w_proj": inp["w_proj"].astype(ml_dtypes.bfloat16),
        "b_proj": inp["b_proj"].astype(np.float32),
        "w1": inp["w1"].astype(ml_dtypes.bfloat16),
        "b1t": b1t,
        "w2": inp["w2"].astype(ml_dtypes.bfloat16),
        "b2": inp["b2"].astype(np.float32),
    }
    if not ln1_triv:
        common["ln1_g"] = inp["ln1_g"].astype(np.float32)
        common["ln1_b"] = inp["ln1_b"].astype(np.float32)
    if not ln2_triv:
        common["ln2_g"] = inp["ln2_g"].astype(np.float32)
        common["ln2_b"] = inp["ln2_b"].astype(np.float32)
    return common


def make_in_maps(inputs):
    inp = {k: np.asarray(v) for k, v in inputs.items()}
    x = inp["x"].astype(np.float32)
    ln1_triv = bool(np.all(inp["ln1_g"] == 1.0) and np.all(inp["ln1_b"] == 0.0))
    ln2_triv = bool(np.all(inp["ln2_g"] == 1.0) and np.all(inp["ln2_b"] == 0.0))
    common = _prep_common(inp, ln1_triv, ln2_triv)
    in_maps = []
    for i in range(NCORES):
        b, half = i // 2, i % 2
        qa, qb = _chunks(half)
        xh = np.concatenate([x[b, qa:qa + 512], x[b, qb:qb + 512]], axis=0)
        m = dict(common)
        m["x_full"] = np.ascontiguousarray(x[b])
        m["x_half"] = np.ascontiguousarray(xh)
        m["masks"] = _make_masks(half)
        in_maps.append(m)
    return in_maps, ln1_triv, ln2_triv


def assemble(results):
    out = np.empty((B, T, C), np.float32)
    for i in range(NCORES):
        b, half = i // 2, i % 2
        qa, qb = _chunks(half)
        o = results[i]["out"]
        out[b, qa:qa + 512] = o[:512]
        out[b, qb:qb + 512] = o[512:]
    return out


def kernel(**inputs):
    in_maps, l1, l2 = make_in_maps(inputs)
    nc, io = build(l1, l2)
    res = run_bass_kernel_spmd(nc, in_maps, list(range(NCORES)))
    return assemble(res.results)


if __name__ == "__main__":
    build()
    print("build ok")



# revision 44
# speedup vs baseline: 42.8071x; 1.0085x over previous
"""Trainium2 Bass kernel for a dense transformer block (B=4,T=2048,C=1024,H=16).

Sharding: 8 cores, zero collectives. Core i handles batch i//2 and a
balanced half of the query tokens (i%2==0 -> chunks [0:512)+[1536:2048),
i%2==1 -> [512:1024)+[1024:1536)). All sharding is done on the host; the
device program is identical on every core (SPMD), only input data differs.

Per-core dataflow (tokens-on-free-axis for all matmul operands):
  LN1 (bn_stats, [tok,C] layout) -> h1 bf16 -> DRAM bounce -> DMA-transpose
  -> h1T [C, tok].  QKV in bf16: kT/qT stacked [H*64, tok] (2 heads per
  128-partition tile), V natural [tok, H*65] with a fused ones column so the
  PV matmul also produces the softmax denominator.  Scores are computed
  pre-transposed sT[tk, tq] = K Q^T so softmax needs no transpose of P and
  no max subtraction (scores are O(1)); exp runs on ACT straight from PSUM
  with the 1/sqrt(hd) scale fused.  Causality at 128-tile granularity with
  per-core host-built multiplicative masks (uniform program, per-core data).
  PV accumulates over tk tiles in PSUM; normalization = one reciprocal of
  the denominator row + gpsimd partition_broadcast + one multiply.
  proj and both FFN matmuls run in float32r (full-rate fp32).
"""

import sys
import numpy as np

for _p in ("/opt/trn_rl_repo", "/root/.axon_site/_ro/trn_rl_repo"):
    if _p not in sys.path:
        sys.path.append(_p)

import ml_dtypes  # noqa: E402
import concourse.bass as bass  # noqa: E402
import concourse.bacc as bacc  # noqa: E402
import concourse.tile as tile  # noqa: E402
from concourse import mybir  # noqa: E402
from concourse.bass_utils import run_bass_kernel_spmd  # noqa: E402
from concourse.masks import make_identity  # noqa: E402

B, T, C, H, HD = 4, 2048, 1024, 16, 64
NCORES = 8
EPS = 1e-5
F32 = mybir.dt.float32
F32R = mybir.dt.float32r
BF16 = mybir.dt.bfloat16
AF = mybir.ActivationFunctionType
ALU = mybir.AluOpType

_CACHE = {}

def _emit_body(nc, tc, io, ln1_triv, ln2_triv):
    # ---------------- long-lived pools ----------------
    def pool(name, bufs, space="SBUF"):
        cm = tc.tile_pool(name=name, bufs=bufs, space=space)
        p = cm.__enter__()
        return cm, p

    cm_singles, singles = pool("singles", 1)
    cm_ln, ln_pool = pool("ln", 3)
    cm_stat, stat_pool = pool("stat", 4)
    cm_small, small = pool("small", 2)
    cm_dram, dram = pool("dram", 1, "DRAM")

    eps_t = singles.tile([128, 1], F32, name="eps")
    nc.vector.memset(eps_t, EPS)
    ident_bf = singles.tile([128, 128], BF16, name="ident_bf")
    make_identity(nc, ident_bf)
    b1t_sb = singles.tile([128, 32], F32, name="b1t_sb")
    nc.gpsimd.dma_start(out=b1t_sb, in_=io["b1t"])

    def bcast_ap(dram_ap):
        # [1024] dram vector -> [128,1024] partition-broadcast AP
        return bass.AP(
            tensor=dram_ap.tensor,
            offset=dram_ap.offset,
            ap=[[0, 128]] + list(dram_ap.ap),
        )

    bproj_sb = singles.tile([128, 1024], F32, name="bproj_sb")
    nc.gpsimd.dma_start(out=bproj_sb, in_=bcast_ap(io["b_proj"]))

    g1_sb = bb1_sb = g2_sb = bb2_sb = None
    if not ln1_triv:
        g1_sb = singles.tile([128, 1024], F32, name="g1_sb")
        nc.gpsimd.dma_start(out=g1_sb, in_=bcast_ap(io["ln1_g"]))
        bb1_sb = singles.tile([128, 1024], F32, name="bb1_sb")
        nc.gpsimd.dma_start(out=bb1_sb, in_=bcast_ap(io["ln1_b"]))
    if not ln2_triv:
        g2_sb = singles.tile([128, 1024], F32, name="g2_sb")
        nc.gpsimd.dma_start(out=g2_sb, in_=bcast_ap(io["ln2_g"]))
        bb2_sb = singles.tile([128, 1024], F32, name="bb2_sb")
        nc.gpsimd.dma_start(out=bb2_sb, in_=bcast_ap(io["ln2_b"]))

    # ---------------- LayerNorm helpers ----------------
    def ln_apply(xt, out_ap, trivial, g_sb, b_sb):
        st = stat_pool.tile([128, 2, 6], F32, tag="bnst", name="bnst")
        for sg in range(2):
            nc.vector.bn_stats(out=st[:, sg, :], in_=xt[:, sg * 512:(sg + 1) * 512])
        mv = stat_pool.tile([128, 2], F32, tag="bnmv", name="bnmv")
        nc.vector.bn_aggr(out=mv, in_=st)
        std = stat_pool.tile([128, 1], F32, tag="bnsd", name="bnsd")
        nc.scalar.activation(out=std, in_=mv[:, 1:2], func=AF.Sqrt, bias=eps_t,
                             scale=1.0)
        rstd = stat_pool.tile([128, 1], F32, tag="bnrs", name="bnrs")
        nc.vector.reciprocal(out=rstd, in_=std)
        if trivial:
            nc.vector.tensor_scalar(
                out=out_ap, in0=xt, scalar1=mv[:, 0:1], scalar2=rstd,
                op0=ALU.subtract, op1=ALU.mult)
        else:
            tmp = ln_pool.tile([128, 1024], F32, tag="lnx", name="lntmp")
            nc.vector.tensor_scalar(
                out=tmp, in0=xt, scalar1=mv[:, 0:1], scalar2=rstd,
                op0=ALU.subtract, op1=ALU.mult)
            nc.vector.tensor_mul(out=tmp, in0=tmp, in1=g_sb)
            nc.vector.tensor_add(out=out_ap, in0=tmp, in1=b_sb)

    def ln_tile(x_src_rows, out_ap, trivial, g_sb, b_sb):
        xt = ln_pool.tile([128, 1024], BF16, tag="lnx", name="lnx")
        nc.sync.dma_start(out=xt, in_=x_src_rows)
        ln_apply(xt, out_ap, trivial, g_sb, b_sb)

    # long-lived result pools, opened bottom-of-stack (LIFO discipline)
    cm_h2t, h2t_pool = pool("h2t", 1)
    h2T = h2t_pool.tile([128, 8, 1024], BF16, name="h2T")
    cm_kt, kt_pool = pool("kt", 8)
    cm_v, v_pool = pool("v", 16)
    cm_qt, qt_pool = pool("qt", 8)
    kT = [kt_pool.tile([128, 2048], BF16, tag="kt", name="kt") for _ in range(8)]
    Vt = [v_pool.tile([128, 16, 65], BF16, tag="vt", name="vt") for _ in range(16)]
    qT = [qt_pool.tile([128, 1024], BF16, tag="qt", name="qt") for _ in range(8)]
    x2d = dram.tile([1024, 1024], BF16, name="x2d")

    # ---------------- Phase 1+2: LN1 -> PE-transpose -> QKV -----------------
    # weight loads are emitted AFTER the first x tiles: the DMA engine is a
    # serial resource and the first LN tiles are on the critical path
    cm_wqkv, wqkv_pool = pool("wqkv", 2)
    wkB = wqkv_pool.tile([128, 8, 1024], BF16, tag="w", name="wkB")
    wvB = wqkv_pool.tile([128, 8, 1024], BF16, tag="w", name="wvB")

    cm_h1t, h1t_pool = pool("h1t", 2)
    cm_pst, ps_tr = pool("ps_tr", 4, "PSUM")
    cm_psq, ps_qkv = pool("ps_qkv", 4, "PSUM")
    cm_h1, h1_pool = pool("h1", 3)

    def ln_transpose(src_rows, dstT, col):
        ht = h1_pool.tile([128, 1024], BF16, tag="h1", name="h1")
        ln_tile(src_rows, ht, ln1_triv, g1_sb, bb1_sb)
        for g in range(2):
            pst = ps_tr.tile([128, 4, 128], BF16, tag="tr", name="pst")
            for c4 in range(4):
                nc.tensor.transpose(
                    out=pst[:, c4, :],
                    in_=ht[:, (g * 4 + c4) * 128:(g * 4 + c4 + 1) * 128],
                    identity=ident_bf)
            nc.scalar.copy(
                out=dstT[:, g * 4:(g + 1) * 4, col * 128:(col + 1) * 128],
                in_=pst)

    h1TA = h1t_pool.tile([128, 8, 1024], BF16, tag="h1t", name="h1TA")
    h1TB = h1t_pool.tile([128, 8, 1024], BF16, tag="h1t", name="h1TB")

    def h1T(c, n):
        # transposed h1 slice [128, 512] for token chunk n (0..3)
        src = h1TA if n < 2 else h1TB
        return src[:, c, (n % 2) * 512:(n % 2 + 1) * 512]

    def h1Tt(c, t):
        # transposed h1 slice [128, 128] for token tile t (0..15)
        src = h1TA if t < 8 else h1TB
        return src[:, c, (t % 8) * 128:(t % 8 + 1) * 128]

    def kt_unit(n, p, late=False):
        pp = ps_pv if late else ps_qkv
        ps = pp.tile([128, 512], F32, tag="pv" if late else "q", name="psk")
        for c in range(8):
            nc.tensor.matmul(
                out=ps, lhsT=wkB[:, c, p * 128:(p + 1) * 128],
                rhs=h1T(c, n), start=(c == 0), stop=(c == 7))
        nc.vector.tensor_copy(out=kT[p][:, n * 512:(n + 1) * 512], in_=ps)

    def v_unit(t, n, late=False):
        pp = ps_pv if late else ps_qkv
        ps = pp.tile([128, 512], F32, tag="pv" if late else "q", name="psv")
        for c in range(8):
            nc.tensor.matmul(
                out=ps, lhsT=h1Tt(c, t),
                rhs=wvB[:, c, n * 512:(n + 1) * 512],
                start=(c == 0), stop=(c == 7))
        nc.vector.tensor_copy(
            out=Vt[t][:, n * 8:(n + 1) * 8, 0:64],
            in_=ps.rearrange("p (h d) -> p h d", d=64))
        if n == 1:
            nc.vector.memset(Vt[t][:, :, 64:65], 1.0)

    def q_unit(n, p, late=False):
        pp = ps_pv if late else ps_qkv
        ps = pp.tile([128, 512], F32, tag="pv" if late else "q", name="psq")
        for c in range(8):
            nc.tensor.matmul(
                out=ps, lhsT=wqB[:, c, p * 128:(p + 1) * 128],
                rhs=h1Th[:, c, n * 512:(n + 1) * 512],
                start=(c == 0), stop=(c == 7))
        nc.vector.tensor_copy(out=qT[p][:, n * 512:(n + 1) * 512], in_=ps)

    # A half (tokens 0-1023): LN+transpose, then K(n=0,1) and V tiles 0-7
    for t in range(8):
        ln_transpose(io["x_full"][t * 128:(t + 1) * 128, :], h1TA, t)
        if t == 3:
            nc.sync.dma_start(
                out=wkB,
                in_=io["wqk"][:, 1024:2048].rearrange("(c p) n -> p c n", p=128))
        if t == 4:
            nc.sync.dma_start(
                out=wvB, in_=io["wv"].rearrange("(c p) n -> p c n", p=128))
        if 4 <= t:
            kt_unit(0, 2 * (t - 4))
            kt_unit(0, 2 * (t - 4) + 1)
    for t in range(8):
        v_unit(t, 0)
        v_unit(t, 1)
        if t % 2 == 1:
            kt_unit(1, t - 1)
            kt_unit(1, t)
    # B half (tokens 1024-2047): transposes, then K(n=2,3)
    for t in range(8, 16):
        ln_transpose(io["x_full"][t * 128:(t + 1) * 128, :], h1TB, t - 8)
        if t == 15:
            for p in range(8):
                kt_unit(2, p)
    for p in range(8):
        kt_unit(3, p)
    # queries: LN+transpose of x_half (wq reuses wk's pool slot), V tiles
    # 8-15 run on PE while the x_half tiles stream in
    h1Th = h1t_pool.tile([128, 8, 1024], BF16, tag="h1t", name="h1Th")
    wqB = wqkv_pool.tile([128, 8, 1024], BF16, tag="w", name="wqB")
    for th in range(8):
        ln_transpose(io["x_half"][th * 128:(th + 1) * 128, :], h1Th, th)
        v_unit(8 + th, 0)
        v_unit(8 + th, 1)
    nc.sync.dma_start(
        out=wqB, in_=io["wqk"][:, 0:1024].rearrange("(c p) n -> p c n", p=128))
    for n in range(2):
        for p in range(8):
            q_unit(n, p)
    cm_h1.__exit__(None, None, None)
    cm_psq.__exit__(None, None, None)
    cm_pst.__exit__(None, None, None)
    cm_h1t.__exit__(None, None, None)
    cm_wqkv.__exit__(None, None, None)

    # ---------------- Phase 3: attention (sw-pipelined) ---------------------
    cm_psh2, ps_h2 = pool("ps_h2", 1, "PSUM")
    cm_wp, wp_pool = pool("wproj", 1)
    wpB = wp_pool.tile([128, 8, 1024], BF16, name="wpB")
    nc.scalar.dma_start(
        out=wpB, in_=io["w_proj"].rearrange("(c p) n -> p c n", p=128))
    cm_att, att_pool = pool("attls", 2)

    cm_masks, masks_pool = pool("masks", 1)

    def load_masks(s):
        m = masks_pool.tile([128, 8, 512], BF16, tag="m", name="masks_sb")
        nc.gpsimd.dma_start(out=m, in_=io["masks"][:, s * 8:(s + 1) * 8, :])
        return m

    cm_pt, pt_pool = pool("pt", 3)
    cm_ast, ast_pool = pool("attst", 2)
    cm_pssc, ps_sc = pool("ps_sc", 2, "PSUM")
    cm_pspv, ps_pv = pool("ps_pv", 3, "PSUM")

    attds = [dram.tile([1024, 512], BF16, name="attd")
             for _ in range(2)]  # [c=h*64+d, 512 tq] per slot
    SCALE = HD ** -0.5

    # proj units (t, n): t<4 available after slot0; t>=4 after slot1.
    # n==1 fuses LN2 + PE-transpose of the finished x2 tile into h2T.
    def proj_unit(t, n, acts, xh):
        ps = ps_pv.tile([128, 512], F32, tag="pv", name="psp")
        for c in range(8):
            nc.tensor.matmul(
                out=ps, lhsT=acts[:, c, :],
                rhs=wpB[:, c, n * 512:(n + 1) * 512],
                start=(c == 0), stop=(c == 7))
        sl = np.s_[:, n * 512:(n + 1) * 512]
        x2t = ln_pool.tile([128, 1024], BF16, tag="x2b", name="x2t") \
            if n == 0 else proj_unit.x2t
        proj_unit.x2t = x2t
        with nc.allow_low_precision(reason="x2 residual kept in bf16"):
            nc.vector.tensor_add(out=x2t[sl], in0=ps, in1=xh[sl])
            nc.vector.tensor_add(out=x2t[sl], in0=x2t[sl], in1=bproj_sb[sl])
        if n == 1:
            nc.sync.dma_start(out=x2d[t * 128:(t + 1) * 128, :], in_=x2t)
            h2 = ln_pool.tile([128, 1024], BF16, tag="lnx", name="h2")
            ln_apply(x2t, h2, ln2_triv, g2_sb, bb2_sb)
            for g in range(2):
                pst = ps_h2.tile([128, 4, 128], BF16, tag="tr2", name="pst2")
                for c4 in range(4):
                    nc.tensor.transpose(
                        out=pst[:, c4, :],
                        in_=h2[:, (g * 4 + c4) * 128:(g * 4 + c4 + 1) * 128],
                        identity=ident_bf)
                nc.scalar.copy(
                    out=h2T[:, g * 4:(g + 1) * 4, t * 128:(t + 1) * 128],
                    in_=pst)

    def load_proj_inputs(t):
        acts = att_pool.tile([128, 8, 128], BF16, tag="attls", name="attls")
        nc.scalar.dma_start(
            out=acts,
            in_=attds[t // 4][:, (t % 4) * 128:(t % 4 + 1) * 128]
            .rearrange("(c p) n -> p c n", p=128))
        xh = ln_pool.tile([128, 1024], BF16, tag="lnx", name="xh2")
        nc.sync.dma_start(out=xh, in_=io["x_half"][t * 128:(t + 1) * 128, :])
        return acts, xh

    proj_jobs = []  # (t, acts, xh) pending per-slot1-hp emission

    def attn_slot(s, msb, proj_ts):
        ntk = 8 if s == 0 else 16
        qc = s * 512
        for hp in range(8):
            pva = [ps_pv.tile([128, 512], F32, tag="pv", name="pv")
                   for _ in range(2)]
            pts = {}
            for tkt in range(ntk):
                ps = ps_sc.tile([128, 2, 512], F32, tag="sc", name="sc")
                for e in range(2):
                    nc.tensor.matmul(
                        out=ps[:, e, :],
                        lhsT=kT[hp][e * 64:(e + 1) * 64,
                                    tkt * 128:(tkt + 1) * 128],
                        rhs=qT[hp][e * 64:(e + 1) * 64, qc:qc + 512],
                        start=True, stop=True)
                pt = pt_pool.tile([128, 2, 512], BF16, tag="pt", name="pt")
                pts[tkt] = pt
                nc.scalar.activation(
                    out=pt.rearrange("p a b -> p (a b)"),
                    in_=ps.rearrange("p a b -> p (a b)"),
                    func=AF.Exp, scale=SCALE)
                if (s == 0) or (tkt >= 8):
                    for e in range(2):
                        nc.vector.tensor_mul(
                            out=pt[:, e, :], in0=pt[:, e, :],
                            in1=msb[:, tkt - 8 * s, :])
                if tkt >= 1:
                    prev = pts.pop(tkt - 1)
                    for e in range(2):
                        nc.tensor.matmul(
                            out=pva[e][0:65, :],
                            lhsT=Vt[tkt - 1][:, 2 * hp + e, :],
                            rhs=prev[:, e, :],
                            start=(tkt - 1 == 0), stop=False)
            last = pts.pop(ntk - 1)
            for e in range(2):
                nc.tensor.matmul(
                    out=pva[e][0:65, :],
                    lhsT=Vt[ntk - 1][:, 2 * hp + e, :],
                    rhs=last[:, e, :],
                    start=False, stop=True)
            for e in range(2):
                rec = small.tile([1, 512], BF16, tag="rec", name="rec")
                with nc.allow_low_precision(reason="softmax denom recip bf16"):
                    nc.vector.reciprocal(out=rec, in_=pva[e][64:65, :])
                bc = small.tile([64, 512], BF16, tag="bc", name="bc")
                nc.gpsimd.partition_broadcast(out_ap=bc, in_ap=rec)
                ast = ast_pool.tile([64, 512], BF16, tag="ast", name="ast")
                nc.vector.tensor_mul(out=ast, in0=pva[e][0:64, :], in1=bc)
                nc.sync.dma_start(
                    out=attds[s][hp * 128 + e * 64:hp * 128 + (e + 1) * 64, :],
                    in_=ast)
            if proj_ts and hp % 2 == 1:
                t = proj_ts[hp // 2]
                acts, xh = load_proj_inputs(t)
                for n in range(2):
                    proj_unit(t, n, acts, xh)

    attn_slot(0, load_masks(0), None)
    attn_slot(1, load_masks(1), [0, 1, 2, 3])

    # proj t4..7
    for t in range(4, 8):
        acts, xh = load_proj_inputs(t)
        for n in range(2):
            proj_unit(t, n, acts, xh)

    cm_pspv.__exit__(None, None, None)
    cm_pssc.__exit__(None, None, None)
    cm_ast.__exit__(None, None, None)
    cm_pt.__exit__(None, None, None)
    cm_masks.__exit__(None, None, None)
    cm_att.__exit__(None, None, None)
    cm_wp.__exit__(None, None, None)
    cm_psh2.__exit__(None, None, None)
    cm_qt.__exit__(None, None, None)
    cm_v.__exit__(None, None, None)
    cm_kt.__exit__(None, None, None)

    # ---------------- Phase 6: FFN ------------------------------------------
    cm_ls, late_singles = pool("lsing", 1)
    b2_sb = late_singles.tile([128, 1024], F32, name="b2_sb")
    nc.gpsimd.dma_start(out=b2_sb, in_=bcast_ap(io["b2"]))
    cm_wb, wbig_pool = pool("wbig", 3)

    def load_w1b(jb):
        w1b = wbig_pool.tile([128, 8, 1024], BF16, tag="wb", name="w1b")
        for hh in range(4):
            nc.sync.dma_start(
                out=w1b[:, hh * 2:(hh + 1) * 2, :],
                in_=io["w1"][hh * 256:(hh + 1) * 256,
                             jb * 1024:(jb + 1) * 1024]
                .rearrange("(c p) n -> p c n", p=128))
        return w1b

    def load_w2b(jb):
        w2b = wbig_pool.tile([128, 8, 1024], BF16, tag="wb", name="w2b")
        for hh in range(4):
            nc.sync.dma_start(
                out=w2b[:, hh * 2:(hh + 1) * 2, :],
                in_=io["w2"][jb * 1024 + hh * 256:jb * 1024 + (hh + 1) * 256, :]
                .rearrange("(j p) n -> p j n", p=128))
        return w2b

    w1b_next = load_w1b(0)
    w2b_next = load_w2b(0)

    cm_psl, ps_late = pool("ps_late", 5, "PSUM")
    # prefetch the FFN residual (x2) tiles; oacc is initialized from them
    cm_xr, xr_pool = pool("xres", 8)
    xres = [xr_pool.tile([128, 1024], BF16, tag="xr", name="xr")
            for _ in range(8)]
    for tg in range(8):
        nc.sync.dma_start(out=xres[tg], in_=x2d[tg * 128:(tg + 1) * 128, :])

    cm_rl, relu_pool = pool("relu", 2)
    cm_oa, oacc_pool = pool("oacc", 8)
    oacc = [oacc_pool.tile([128, 1024], F32, tag="oacc", name="oacc")
            for _ in range(8)]
    for jb in range(4):
        w1b = w1b_next
        relu_b = relu_pool.tile([128, 8, 2, 512], BF16, tag="rl", name="rl")
        for pas in range(2):
            tok0 = pas * 512
            for j in range(8):
                ps = ps_late.tile([128, 512], F32, tag="l", name="psf1")
                for c in range(8):
                    nc.tensor.matmul(
                        out=ps,
                        lhsT=w1b[:, c, j * 128:(j + 1) * 128],
                        rhs=h2T[:, c, tok0:tok0 + 512],
                        start=(c == 0), stop=(c == 7))
                nc.scalar.activation(
                    out=relu_b[:, j, pas, :], in_=ps, func=AF.Relu,
                    bias=b1t_sb[:, jb * 8 + j:jb * 8 + j + 1], scale=1.0)
        w2b = w2b_next
        if jb < 3:
            w1b_next = load_w1b(jb + 1)
        for pas in range(2):
            for tl in range(4):
                tg = pas * 4 + tl
                for n in range(2):
                    ps = ps_late.tile([128, 512], F32, tag="l", name="psf2")
                    for j in range(8):
                        nc.tensor.matmul(
                            out=ps,
                            lhsT=relu_b[:, j, pas, tl * 128:(tl + 1) * 128],
                            rhs=w2b[:, j, n * 512:(n + 1) * 512],
                            start=(j == 0), stop=(j == 7))
                    sl = np.s_[:, n * 512:(n + 1) * 512]
                    if jb == 0:
                        nc.vector.tensor_add(out=oacc[tg][sl], in0=ps,
                                             in1=xres[tg][sl])
                    else:
                        nc.vector.tensor_add(out=oacc[tg][sl], in0=oacc[tg][sl],
                                             in1=ps)
                if jb == 3:
                    nc.vector.tensor_add(out=oacc[tg], in0=oacc[tg], in1=b2_sb)
                    nc.sync.dma_start(out=io["out"][tg * 128:(tg + 1) * 128, :],
                                      in_=oacc[tg])
            if jb < 3 and pas == 0:
                w2b_next = load_w2b(jb + 1)

    cm_oa.__exit__(None, None, None)
    cm_rl.__exit__(None, None, None)
    cm_xr.__exit__(None, None, None)
    cm_psl.__exit__(None, None, None)
    cm_wb.__exit__(None, None, None)
    cm_ls.__exit__(None, None, None)
    cm_h2t.__exit__(None, None, None)
    cm_dram.__exit__(None, None, None)
    cm_small.__exit__(None, None, None)
    cm_stat.__exit__(None, None, None)
    cm_ln.__exit__(None, None, None)
    cm_singles.__exit__(None, None, None)


def build(ln1_triv=True, ln2_triv=True):
    key = (ln1_triv, ln2_triv)
    if key in _CACHE:
        return _CACHE[key]
    nc = bacc.Bacc("TRN2", target_bir_lowering=False, debug=False,
                   num_devices=NCORES)
    io = {}

    def din(name, shape, dt):
        io[name] = nc.dram_tensor(name, list(shape), dt, kind="ExternalInput").ap()

    din("x_full", (2048, 1024), BF16)
    din("x_half", (1024, 1024), BF16)
    din("wqk", (1024, 2048), BF16)
    din("wv", (1024, 1024), BF16)
    din("w_proj", (1024, 1024), BF16)
    din("b_proj", (1024,), F32)
    din("w1", (1024, 4096), BF16)
    din("b1t", (128, 32), F32)
    din("w2", (4096, 1024), BF16)
    din("b2", (1024,), F32)
    din("masks", (128, 16, 512), BF16)
    if not ln1_triv:
        din("ln1_g", (1024,), F32)
        din("ln1_b", (1024,), F32)
    if not ln2_triv:
        din("ln2_g", (1024,), F32)
        din("ln2_b", (1024,), F32)
    io["out"] = nc.dram_tensor("out", [1024, 1024], F32, kind="ExternalOutput").ap()

    with tile.TileContext(nc) as tc:
        _emit_body(nc, tc, io, ln1_triv, ln2_triv)
    nc.compile()
    _CACHE[key] = (nc, io)
    return nc, io


def _chunks(half):
    if half == 0:
        return (0, 1536)   # chunk A base, chunk B base
    return (512, 1024)


def _make_masks(half):
    """[128, 16, 512] bf16: m 0-7 = slot0 tiles (queries=chunkA),
    m 8-15 = slot1 tiles 8-15 (queries=chunkB)."""
    qa, qb = _chunks(half)
    out = np.zeros((128, 16, 512), np.float32)
    tk_l = np.arange(128)[:, None]
    tq_l = np.arange(512)[None, :]
    for m in range(8):
        out[:, m, :] = ((m * 128 + tk_l) <= (qa + tq_l))
    for m in range(8, 16):
        out[:, m, :] = ((m * 128 + tk_l) <= (qb + tq_l))
    return out.astype(ml_dtypes.bfloat16)


def _prep_common(inp, ln1_triv, ln2_triv):
    wq_f = np.ascontiguousarray(inp["wq"].transpose(1, 0, 2).reshape(C, C))
    wk_f = np.ascontiguousarray(inp["wk"].transpose(1, 0, 2).reshape(C, C))
    wv_f = np.ascontiguousarray(inp["wv"].transpose(1, 0, 2).reshape(C, C))
    wqk = np.concatenate([wq_f, wk_f], axis=1).astype(ml_dtypes.bfloat16)
    b1t = np.ascontiguousarray(inp["b1"].reshape(32, 128).T).astype(np.float32)
    common = {
        "wqk": wqk,
        "wv": wv_f.astype(ml_dtypes.bfloat16),
        "w_proj": inp["w_proj"].astype(ml_dtypes.bfloat16),
        "b_proj": inp["b_proj"].astype(np.float32),
        "w1": inp["w1"].astype(ml_dtypes.bfloat16),
        "b1t": b1t,
        "w2": inp["w2"].astype(ml_dtypes.bfloat16),
        "b2": inp["b2"].astype(np.float32),
    }
    if not ln1_triv:
        common["ln1_g"] = inp["ln1_g"].astype(np.float32)
        common["ln1_b"] = inp["ln1_b"].astype(np.float32)
    if not ln2_triv:
        common["ln2_g"] = inp["ln2_g"].astype(np.float32)
        common["ln2_b"] = inp["ln2_b"].astype(np.float32)
    return common


def make_in_maps(inputs):
    inp = {k: np.asarray(v) for k, v in inputs.items()}
    x = inp["x"].astype(np.float32)
    ln1_triv = bool(np.all(inp["ln1_g"] == 1.0) and np.all(inp["ln1_b"] == 0.0))
    ln2_triv = bool(np.all(inp["ln2_g"] == 1.0) and np.all(inp["ln2_b"] == 0.0))
    common = _prep_common(inp, ln1_triv, ln2_triv)
    in_maps = []
    for i in range(NCORES):
        b, half = i // 2, i % 2
        qa, qb = _chunks(half)
        xh = np.concatenate([x[b, qa:qa + 512], x[b, qb:qb + 512]], axis=0)
        m = dict(common)
        m["x_full"] = np.ascontiguousarray(x[b]).astype(ml_dtypes.bfloat16)
        m["x_half"] = np.ascontiguousarray(xh).astype(ml_dtypes.bfloat16)
        m["masks"] = _make_masks(half)
        in_maps.append(m)
    return in_maps, ln1_triv, ln2_triv


def assemble(results):
    out = np.empty((B, T, C), np.float32)
    for i in range(NCORES):
        b, half = i // 2, i % 2
        qa, qb = _chunks(half)
        o = results[i]["out"]
        out[b, qa:qa + 512] = o[:512]
        out[b, qb:qb + 512] = o[512:]
    return out


def kernel(**inputs):
    in_maps, l1, l2 = make_in_maps(inputs)
    nc, io = build(l1, l2)
    res = run_bass_kernel_spmd(nc, in_maps, list(range(NCORES)))
    return assemble(res.results)


if __name__ == "__main__":
    build()
    print("build ok")

